# revision 5
# baseline (speedup 1.0000x reference)
"""Trainium2 Bass kernel for nn_Amplified_PatternMixer.

Computation:
  out[b, h, m1, m2] = mixed_pattern[h, m1, m2] + alpha[h] * nrm[b, m2]
where
  nrm[b, m] = || mean_{hw}(x[b*57+m, :, h, w]) ||_2   over channels
  mixed_pattern = tiny 57x57 graph-normalized pattern (from 5x7x7 params).

The memory-bound part (streaming x: [1824, 256, 14, 14] f32, ~366 MB) runs
on 8 NeuronCores, data-parallel over rows (228 rows/core).  Each core:
  - DMAs its row shard in channel-chunks to SBUF (rows on partitions)
  - reduce_sum over the 196-elem HW dim per channel -> channel sums
  - square + reduce over channels, sqrt, scale by 1/196 -> per-row norm
The tiny pattern-mixer math (57x57, a few thousand flops) runs on host.
"""

import os

import numpy as np

import concourse.bacc as bacc
import concourse.bass as bass
import concourse.mybir as mybir
import concourse.tile as tile
from concourse.bass_utils import run_bass_kernel_spmd

# Problem constants (hardcoded; kernel.py must be self-contained).
NUM_BASIC = 5
NUM_MIXED = 4
NUM_FRAME = 8
NUM_NODES = 7
NUM_SAMPLES = 8
M = 1 + NUM_NODES * NUM_FRAME  # 57

N_CORES = 8
B = 32
C = 256
HW = 196  # 14*14
ROWS_TOTAL = B * M          # 1824
ROWS_PER_CORE = ROWS_TOTAL // N_CORES  # 228
CW = C * HW                 # 50176 floats per row
G = 64                      # channels per SBUF chunk
N_CHUNKS = C // G
ROW_GROUPS = [(0, 128), (128, ROWS_PER_CORE - 128)]

LAST_RESULT = None
_NC_CACHE = None


def _build_nc():
    nc = bacc.Bacc(None)
    x = nc.declare_dram_parameter(
        "x", [ROWS_PER_CORE, CW], mybir.dt.float32, isOutput=False
    )
    out = nc.declare_dram_parameter(
        "out", [ROWS_PER_CORE, 1], mybir.dt.float32, isOutput=True
    )
    with tile.TileContext(nc) as tc:
        with (
            tc.tile_pool(name="xt_pool", bufs=3) as xp,
            tc.tile_pool(name="acc_pool", bufs=2) as accp,
            tc.tile_pool(name="res_pool", bufs=2) as resp,
        ):
            for r0, P in ROW_GROUPS:
                cs = accp.tile([128, C], mybir.dt.float32, tag="cs")
                for ci in range(N_CHUNKS):
                    xt = xp.tile([128, G * HW], mybir.dt.float32, tag="xt")
                    nc.gpsimd.dma_start(
                        out=xt[:P],
                        in_=x[r0 : r0 + P, ci * G * HW : (ci + 1) * G * HW],
                    )
                    nc.vector.reduce_sum(
                        cs[:P, ci * G : (ci + 1) * G],
                        xt[:P].rearrange("p (g w) -> p g w", w=HW),
                        axis=mybir.AxisListType.X,
                    )
                sq = accp.tile([128, C], mybir.dt.float32, tag="sq")
                nc.vector.tensor_mul(sq[:P], cs[:P], cs[:P])
                ss = resp.tile([128, 1], mybir.dt.float32, tag="ss")
                nc.vector.reduce_sum(ss[:P], sq[:P], axis=mybir.AxisListType.X)
                nrm = resp.tile([128, 1], mybir.dt.float32, tag="nrm")
                # nrm = sqrt(ss / 196^2) = sqrt(ss) / 196
                nc.scalar.activation(
                    nrm[:P],
                    ss[:P],
                    mybir.ActivationFunctionType.Sqrt,
                    scale=1.0 / float(HW * HW),
                )
                nc.gpsimd.dma_start(out=out[r0 : r0 + P, :], in_=nrm[:P])
    nc.finalize()
    return nc


def _get_nc():
    global _NC_CACHE
    if _NC_CACHE is None:
        _NC_CACHE = _build_nc()
    return _NC_CACHE


def _zero_mask():
    mask = np.ones((M, M), dtype=np.float64)
    for i in range(NUM_SAMPLES):
        r = (1 + i) * NUM_NODES
        for c in range(1, M):
            if c % NUM_NODES != 0 and (c - 1) // NUM_NODES != i:
                mask[r, c] = 0.0
    return mask


def _pattern_mixer_np(mat, sigma, lin_w, lin_b, mixed_mat):
    mat = np.asarray(mat, np.float64)            # [5, 7, 7]
    sigma = np.asarray(sigma, np.float64)        # [4, 5, 1]
    lin_w = np.asarray(lin_w, np.float64)        # [4, 5]
    lin_b = np.asarray(lin_b, np.float64)        # [4]
    mixed_mat = np.asarray(mixed_mat, np.float64)  # [4, 57, 57]

    T2 = 2 * NUM_FRAME - 1  # 15
    dist = np.abs(np.arange(T2, dtype=np.float64) - (NUM_FRAME - 1))
    te = (1.0 / (np.sqrt(2.0 * np.pi) * sigma)) * np.exp(
        -(dist**2) / (2.0 * sigma**2)
    )  # [4, 5, 15]
    ce = 1.0 / (1.0 + np.exp(-te))
    mixed = (
        np.einsum("hbt,bnm,hb->hntm", ce, mat, lin_w)
        + lin_b[:, None, None, None]
    )
    mixed = np.maximum(mixed, 0.0).reshape(NUM_MIXED, NUM_NODES, T2 * NUM_NODES)
    blocks = [
        mixed[
            :,
            :,
            NUM_NODES * (NUM_SAMPLES - 1 - i) : NUM_NODES * (2 * NUM_SAMPLES - 1 - i),
        ]
        for i in range(NUM_SAMPLES)
    ]
    add_block = np.concatenate(blocks, axis=1)  # [4, 56, 56]
    mm = mixed_mat.copy()
    mm[:, 1:, 1:] += add_block
    mm *= _zero_mask()[None]
    deg = np.maximum(mm.sum(axis=2), 1.0) ** -0.5  # [4, 57]
    return (deg[:, :, None] * mm * deg[:, None, :]).astype(np.float32)


def kernel(mat, x, sigma, lin_w, lin_b, mixed_mat, alpha):
    global LAST_RESULT
    x = np.ascontiguousarray(np.asarray(x, dtype=np.float32))
    xs = x.reshape(ROWS_TOTAL, CW)
    in_maps = [
        {"x": xs[i * ROWS_PER_CORE : (i + 1) * ROWS_PER_CORE]} for i in range(N_CORES)
    ]
    nc = _get_nc()
    res = run_bass_kernel_spmd(nc, in_maps, core_ids=list(range(N_CORES)))
    LAST_RESULT = res
    norms = np.concatenate([r["out"][:, 0] for r in res.results])  # [1824]
    nrm = norms.reshape(B, M)

    mp = _pattern_mixer_np(mat, sigma, lin_w, lin_b, mixed_mat)  # [4, 57, 57] f32
    alpha = np.asarray(alpha, np.float32).reshape(1, NUM_MIXED, 1, 1)
    out = mp[None] + alpha * nrm[:, None, None, :]
    return np.ascontiguousarray(out.astype(np.float32))
